# revision 3
# baseline (speedup 1.0000x reference)
"""Channel-attention (transposed attention) Trainium2 Bass kernel.

Reference computation (per batch b of 8, one NeuronCore each):
    xt   = x[b].reshape(C, N).T                    # [N, C], N = 64*64 = 4096
    qkv  = xt @ w_qkv                              # [N, 3C]
    q, k, v : per-head [N, hd], nh=8, hd=64
    logits_h = k_h.T @ v_h                         # [hd, hd]
    attn_h   = softmax(scale * logits_h, axis=-1)  # scale = hd**-0.5 = 1/8
    out_h    = q_h @ attn_h.T                      # [N, hd]
    y[b] = (concat_h(out_h) @ w_proj + b_proj).T   # [C, N]

Sharding: data-parallel over batch, 1 batch item per core, no collectives.

Algebraic fusion: everything downstream of the softmax is LINEAR in x, so
the q projection, the attention apply, and the output projection collapse
into one [C, C] matrix applied directly to x:

    out^T_h = attn_h @ W_qh^T @ x      (W_qh = w_qkv[:, q cols of head h])
    y^T     = w_proj^T @ out^T + b
            = G @ x + b,   G = w_proj^T @ stack_h(attn_h @ W_qh^T)

This removes the q^T matmuls (128 FD=512 MMs) and the attention-apply
matmuls (32 FD=512 MMs) of the direct formulation, replacing them with
16 PE transposes of w_q (for W_q^T), 4 M-build MMs and 16 G-build MMs.
x stays resident in SBUF (8 MB) and is re-read for the final G @ x pass,
so HBM traffic is unchanged (x once in, y once out, weights once).

On-core layout: x[b] is [C, N] in DRAM; k/v are computed with tokens on
partitions using x token-tiles as the stationary operand (lhsT) — no
activation transpose anywhere. The per-head [64,64] softmax matrices are
packed two heads per 128 partitions as block-diagonal matrices, so the
M-build and G-build run with full 128-row contraction.

The big (free-dim 512) matmuls use float32r (fp32 bytes, FP22 multiply)
— 4x faster than true fp32 at free-dim >= 256, ~6e-5 relative element
precision. The small per-head logit matmuls (free-dim 64) stay exact
fp32, which also permits the tile_position col-64 packing that runs odd
heads into partitions 64:128.
"""

import numpy as np

import concourse.bass as bass
import concourse.mybir as mybir
import concourse.tile as tile
from concourse import bass_utils

F32 = mybir.dt.float32
F32R = mybir.dt.float32r
AF = mybir.ActivationFunctionType

# Problem shape (hardcoded per contest contract).
B = 8
C = 512
H = W = 64
N = H * W            # 4096 tokens per batch
NH = 8               # heads
HD = C // NH         # 64
SCALE = HD ** -0.5   # 1/8
KC = C // 128        # 4 contraction chunks of 128 channels
NS = 8               # n-slices of 512 tokens
SL = N // NS         # 512
TT = SL // 128       # 4 token tiles of 128 per slice
HP = NH // 2         # 4 head pairs


def _r(ap):
    return ap.bitcast(F32R)


def _split_multi_waits(nc, max_waits=1):
    """The walrus build in this container encodes at most one sync-wait
    command per instruction (setupSyncWait raises "Too many sync wait
    commands" otherwise — the Tile kernel-tail drain carries several).
    Hoist excess waits onto same-engine NOPs immediately preceding the
    instruction; engine-FIFO order preserves the semantics."""
    n_split = 0
    for bb in nc.main_func.blocks:
        new_insts = []
        for ins in bb.instructions:
            si = ins.sync_info
            waits = list(si.on_wait) if si and si.on_wait else []
            if len(waits) > max_waits:
                extra, keep = waits[:-max_waits], waits[-max_waits:]
                while extra:
                    chunk, extra = extra[:max_waits], extra[max_waits:]
                    nop = mybir.InstNoOp(
                        name=nc.get_next_instruction_name(),
                        ins=[], outs=[],
                        engine=ins.engine,
                        sync_info=mybir.SyncInfo(on_wait=chunk, on_update=[]),
                    )
                    nc.register_instruction(nop)
                    new_insts.append(nop)
                    n_split += 1
                si.on_wait = keep
            new_insts.append(ins)
        bb.instructions[:] = new_insts
    return n_split


def build_nc(reps=1, phases='full'):
    nc = bass.Bass("TRN2", debug=False, num_devices=B)

    x_t = nc.dram_tensor("x", [C, N], F32, kind="ExternalInput")
    wq_t = nc.dram_tensor("w_qkv", [C, 3 * C], F32, kind="ExternalInput")
    wp_t = nc.dram_tensor("w_proj", [C, C], F32, kind="ExternalInput")
    bp_t = nc.dram_tensor("b_proj", [C, 1], F32, kind="ExternalInput")
    y_t = nc.dram_tensor("y", [C, N], F32, kind="ExternalOutput")
    id_t = nc.inline_tensor(np.eye(128, dtype=np.float32), name="id128")

    xd, wqd, wpd, bpd, yd = x_t.ap(), wq_t.ap(), wp_t.ap(), bp_t.ap(), y_t.ap()

    with tile.TileContext(nc) as tc:
        with (
            tc.tile_pool(name="const", bufs=1) as cpool,
            tc.tile_pool(name="xres", bufs=1) as xpool,
            tc.tile_pool(name="soft", bufs=1) as spool,
        ):
            id_sb = cpool.tile([128, 128], F32, tag="id")
            nc.sync.dma_start(id_sb[:], id_t.ap()[:, :])

            # ---- slice-0 x tiles + the k-section of w_qkv lead the DMA
            # queue, interleaved per chunk so the first kv accumulation
            # group can start after ~0.5 MB of traffic ----
            x_sb = [[xpool.tile([128, SL], F32R, name=f"x{s}_{k}",
                                tag=f"x{s}_{k}") for k in range(KC)]
                    for s in range(NS)]
            wq_sb = [cpool.tile([128, 3 * C], F32R, name=f"wq{k}", tag=f"wq{k}")
                     for k in range(KC)]
            for k in range(KC):
                r = slice(k * 128, (k + 1) * 128)
                nc.sync.dma_start(x_sb[0][k][:], _r(xd[r, 0:SL]))
                nc.sync.dma_start(wq_sb[k][:, 512:1024], _r(wqd[r, 512:1024]))
            for k in range(KC):
                r = slice(k * 128, (k + 1) * 128)
                nc.sync.dma_start(wq_sb[k][:, 1024:1536], _r(wqd[r, 1024:1536]))
            for k in range(KC):
                r = slice(k * 128, (k + 1) * 128)
                nc.sync.dma_start(wq_sb[k][:, 0:512], _r(wqd[r, 0:512]))

            wp_sb = [cpool.tile([128, C], F32R, name=f"wp{k}", tag=f"wp{k}")
                     for k in range(KC)]
            bp_sb = [cpool.tile([128, 1], F32, name=f"bp{k}", tag=f"bp{k}")
                     for k in range(KC)]
            wqqT_sb = [cpool.tile([128, C], F32R, name=f"wqqT{j}", tag=f"wqqT{j}")
                       for j in range(KC)]
            gT_sb = [cpool.tile([128, C], F32R, name=f"gT{c}", tag=f"gT{c}")
                     for c in range(KC)]
            m_sb = [cpool.tile([128, C], F32R, name=f"m{p}", tag=f"m{p}")
                    for p in range(HP)]

            for _rep in range(reps):
                _build_one_pass(nc, tc, spool, wq_sb, wp_sb, bp_sb, id_sb,
                                wqqT_sb, gT_sb, m_sb, x_sb, xd, yd, wpd, bpd,
                                first_rep=(_rep == 0), phases=phases)
    _split_multi_waits(nc)
    return nc


def _build_one_pass(nc, tc, spool, wq_sb, wp_sb, bp_sb, id_sb, wqqT_sb,
                    gT_sb, m_sb, x_sb, xd, yd, wpd, bpd, first_rep=True,
                    phases="full"):
    # phases: prefix gating for attribution benchmarks
    lvl = ["dma", "qkv", "logits", "soft", "attn", "full"].index(phases)

    # softmax logits accumulators: heads packed 2-per-128-partitions,
    # even heads (par=0) in lg_a partitions 0:64, odd heads (par=1)
    # in lg_b partitions 64:128 (separate banks so each partition
    # half runs its own clean psum accumulation group):
    # lg_{a,b}[64*par + d, hp*64 + e] = logits of head (2*hp + par)
    with tc.tile_pool(name="lgp", bufs=1, space="PSUM") as lgpool:
        lg_a = lgpool.tile([128, HP * HD], F32, tag="lg_a")
        lg_b = lgpool.tile([128, HP * HD], F32, tag="lg_b")
        lg = [lg_a, lg_b]

        # ================= Phase A: KV + logit accumulation =========
        with (
            tc.tile_pool(name="kvs", bufs=3) as kvpool,
            tc.tile_pool(name="kvp", bufs=2, space="PSUM") as kvpsum,
            tc.tile_pool(name="wtp", bufs=2, space="PSUM") as wtpsum,
        ):
            for ns in range(NS):
                if first_rep and ns >= 1:
                    # stream the rest of x in slice order, behind the
                    # startup-critical loads above
                    for k in range(KC):
                        nc.sync.dma_start(
                            x_sb[ns][k][:],
                            _r(xd[k * 128:(k + 1) * 128,
                                  ns * SL:(ns + 1) * SL]),
                        )
                    if ns == 1:
                        # deferred weight loads: w_proj/b_proj are first
                        # needed at G-build / phase D
                        for k in range(KC):
                            r = slice(k * 128, (k + 1) * 128)
                            nc.sync.dma_start(wp_sb[k][:], _r(wpd[r, :]))
                            nc.sync.dma_start(bp_sb[k][:], bpd[r, :])
                xs = x_sb[ns]
                if lvl < 1:
                    continue
                # --- k,v token tiles + logit accumulation ---
                for t in range(TT):
                    kvp = kvpsum.tile([128, 2 * C], F32, tag="kv_ps")
                    for k in range(KC):
                        xk = xs[k][:, t * 128:(t + 1) * 128]
                        nc.tensor.matmul(
                            kvp[:, 0:512], xk, wq_sb[k][:, 512:1024],
                            start=(k == 0), stop=(k == KC - 1),
                        )
                        nc.tensor.matmul(
                            kvp[:, 512:1024], xk, wq_sb[k][:, 1024:1536],
                            start=(k == 0), stop=(k == KC - 1),
                        )
                    kv_sb = kvpool.tile([128, 2 * C], F32, tag="kv_sb")
                    nc.vector.tensor_copy(kv_sb[:], kvp[:])
                    if lvl < 2:
                        continue
                    first = ns == 0 and t == 0
                    last = ns == NS - 1 and t == TT - 1
                    for h in range(NH):
                        hp, par = divmod(h, 2)
                        # start=True marks the whole 2KB psum bank
                        # pending-zero, so only head 0/1 of the
                        # first tile starts each bank's group and
                        # only head 6/7 of the last tile stops it;
                        # the other heads' first write lands on
                        # still-pending bytes and overwrites.
                        nc.tensor.matmul(
                            lg[par][par * 64:(par + 1) * 64,
                                    hp * 64:(hp + 1) * 64],
                            kv_sb[:, h * 64:(h + 1) * 64],
                            kv_sb[:, 512 + h * 64:512 + (h + 1) * 64],
                            start=first and h < 2,
                            stop=last and h >= NH - 2,
                        )

                if ns == 1 and first_rep:
                    # W_q^T via PE transposes — wq q-section has landed
                    # by now; runs in the PE stream between kv groups.
                    # wqqT[j][:, k*128:(k+1)*128] = wq[k chunk, j blk]^T
                    for j in range(KC):
                        for k in range(KC):
                            wt = wtpsum.tile([128, 128], F32, tag="wt")
                            nc.tensor.transpose(
                                wt[:],
                                wq_sb[k][:, j * 128:(j + 1) * 128].bitcast(F32),
                                id_sb[:],
                            )
                            nc.scalar.activation(
                                wqqT_sb[j][:, k * 128:(k + 1) * 128],
                                wt[:], AF.Copy,
                            )

        if lvl < 3:
            return
        # ================= Phase B: softmax on [64,64] logits =======
        # BD[hp]: block-diag exp(scale*(logits-max)) for head pair hp
        bd = [spool.tile([128, 128], F32, name=f"bd{p}", tag=f"bd{p}")
              for p in range(HP)]
        mx = spool.tile([128, HP], F32, tag="mx")
        bias = spool.tile([128, HP], F32, tag="bias")
        ssum = spool.tile([128, HP], F32, tag="ssum")
        recip = spool.tile([128, HP], F32, tag="recip")
        at_sb = [spool.tile([128, 128], F32R, name=f"at{p}", tag=f"at{p}")
                 for p in range(HP)]

        for p in range(HP):
            nc.gpsimd.memset(bd[p][:], 0.0)
        for p in range(HP):
            csl = slice(p * 64, (p + 1) * 64)
            for par in range(2):
                psl = slice(par * 64, (par + 1) * 64)
                nc.vector.reduce_max(
                    mx[psl, p:p + 1], lg[par][psl, csl],
                    axis=mybir.AxisListType.X,
                )
        nc.vector.tensor_scalar_mul(bias[:], mx[:], -SCALE)
        for p in range(HP):
            csl = slice(p * 64, (p + 1) * 64)
            for par in range(2):
                psl = slice(par * 64, (par + 1) * 64)
                # diag block (par==0 -> cols 0:64, par==1 -> cols 64:128)
                nc.scalar.activation(
                    bd[p][psl, psl], lg[par][psl, csl], AF.Exp,
                    bias=bias[psl, p:p + 1], scale=SCALE,
                )
                nc.vector.reduce_sum(
                    ssum[psl, p:p + 1], bd[p][psl, psl],
                    axis=mybir.AxisListType.X,
                )
        nc.vector.reciprocal(recip[:], ssum[:])

    # transpose each block-diag exp matrix on the PE: at = bd^T
    with tc.tile_pool(name="bdt", bufs=2, space="PSUM") as bdtpool:
        for p in range(HP):
            bdt = bdtpool.tile([128, 128], F32, tag="bdt")
            nc.tensor.transpose(bdt[:], bd[p][:], id_sb[:])
            nc.vector.tensor_copy(at_sb[p][:], bdt[:])

    # ---- M build: M[pair p rows, :] = blockdiag(attn) @ W_q^T ----
    # out = at_sb[p]^T @ wqqT[p block] = blockdiag(exp_h) @ W_q^T rows,
    # then row-normalize by the softmax sums (diag(1/s) folded here).
    with tc.tile_pool(name="mps", bufs=2, space="PSUM") as mpsum:
        for p in range(HP):
            mp = mpsum.tile([128, C], F32, tag="m_ps")
            nc.tensor.matmul(mp[:], at_sb[p][:], wqqT_sb[p][:],
                             start=True, stop=True)
            nc.vector.tensor_scalar_mul(
                m_sb[p][:], mp[:], recip[:, p:p + 1])

    # ---- G^T build: G^T[ic chunk c, :] = sum_dc M[dc, c blk]^T @ wp[dc]
    with tc.tile_pool(name="gps", bufs=2, space="PSUM") as gpsum:
        for cc in range(KC):
            gp = gpsum.tile([128, C], F32, tag="g_ps")
            for dc in range(KC):
                nc.tensor.matmul(
                    gp[:], m_sb[dc][:, cc * 128:(cc + 1) * 128],
                    wp_sb[dc][:],
                    start=(dc == 0), stop=(dc == KC - 1),
                )
            nc.scalar.activation(gT_sb[cc][:], gp[:], AF.Copy)

    if lvl < 4:
        return
    # ================= Phase D: y^T = G @ x + b =====================
    with (
        tc.tile_pool(name="ys", bufs=3) as ypool,
        tc.tile_pool(name="yp", bufs=3, space="PSUM") as ypsum,
    ):
        for ns in range(NS):
            nsl = slice(ns * SL, (ns + 1) * SL)
            for co in range(KC):
                yp = ypsum.tile([128, SL], F32, tag="y_ps")
                for k in range(KC):
                    nc.tensor.matmul(
                        yp[:],
                        gT_sb[k][:, co * 128:(co + 1) * 128],
                        x_sb[ns][k][:],
                        start=(k == 0),
                        stop=(k == KC - 1),
                    )
                ysb = ypool.tile([128, SL], F32, tag="y_sb")
                nc.scalar.activation(
                    ysb[:], yp[:], AF.Identity,
                    bias=bp_sb[co][:, 0:1], scale=1.0,
                )
                nc.sync.dma_start(
                    yd[co * 128:(co + 1) * 128, nsl], ysb[:]
                )


_NC_CACHE = None


def kernel(x, w_qkv, w_proj, b_proj, num_heads):
    x = np.ascontiguousarray(np.asarray(x, dtype=np.float32))
    w_qkv = np.ascontiguousarray(np.asarray(w_qkv, dtype=np.float32))
    w_proj = np.ascontiguousarray(np.asarray(w_proj, dtype=np.float32))
    b_proj = np.ascontiguousarray(np.asarray(b_proj, dtype=np.float32))
    assert int(num_heads) == NH
    assert x.shape == (B, C, H, W)

    xs = x.reshape(B, C, N)
    bp2 = b_proj.reshape(C, 1)
    in_maps = [
        {"x": xs[b], "w_qkv": w_qkv, "w_proj": w_proj, "b_proj": bp2}
        for b in range(B)
    ]
    global _NC_CACHE
    if _NC_CACHE is None:
        _NC_CACHE = build_nc()
    res = bass_utils.run_bass_kernel_spmd(_NC_CACHE, in_maps, list(range(B)))
    y = np.stack([res.results[b]["y"] for b in range(B)])
    return y.reshape(B, C, H, W).astype(np.float32)


if __name__ == "__main__":
    nc = build_nc()
    n_inst = sum(len(bb.instructions) for bb in nc.main_func.blocks)
    print(f"built OK, {n_inst} instructions")
